# revision 41
# baseline (speedup 1.0000x reference)
import numpy as np

NV = 100000
NTOT = 200000
C = 2048
CPC = 256            # clusters per core
NCORES = 8
CHUNKS = 196         # output chunks of 128 ids per core
IDS_PER_CORE = CHUNKS * 128          # 25088
TPAD = NCORES * IDS_PER_CORE         # 200704 padded id space
SEND_REAL = CPC * 128                # 32768 h rows per core
SEND_ROWS = SEND_REAL + 128          # + zero block
GAMMA = 1.0
SCALE = 8.0          # sqrt(64)

_cache = {}


def _build(BPC):
    import concourse.bass as bass
    import concourse.mybir as mybir
    import concourse.tile as tile
    import concourse.bacc as bacc
    from concourse.masks import make_identity

    f32 = mybir.dt.float32
    f16 = mybir.dt.float16
    i32 = mybir.dt.int32
    NBLK = CHUNKS * BPC

    nc = bacc.Bacc("TRN2", target_bir_lowering=False, debug=False)
    ptab_sh = nc.dram_tensor("ptab_sh", [IDS_PER_CORE, 65], f16, kind="ExternalInput")
    xg_off = nc.dram_tensor("xg_off", [128, CPC], i32, kind="ExternalInput")
    mrg_off = nc.dram_tensor("mrg_off", [128, NBLK], i32, kind="ExternalInput")
    ids_u8 = nc.dram_tensor("ids_u8", [128, NBLK], mybir.dt.uint8, kind="ExternalInput")
    B_T = nc.dram_tensor("B_T", [64, 64], f32, kind="ExternalInput")
    W_VT = nc.dram_tensor("W_VT", [64, 64], f32, kind="ExternalInput")
    W_oT = nc.dram_tensor("W_oT", [65, 64], f32, kind="ExternalInput")
    bo_rep = nc.dram_tensor("bo_rep", [128, 64], f32, kind="ExternalInput")
    # outputs: int8 delta + per-row fp16 scale, replicated on every core by a
    # final AllGather so the host fetches one single-stream copy (per-shard
    # D2H has heavy overhead). Host reconstructs x + q*scale in f32.
    # wide-row, 4-byte-dtype output shapes: the tunnel's D2H throughput is
    # per-element, so ship the int8/f16 payload bitcast as int32/f32
    i8 = mybir.dt.int8
    out_qa = nc.dram_tensor("out_qa", [NV // 800, 12800], i32, kind="ExternalOutput")
    out_qb = nc.dram_tensor("out_qb", [(NTOT - NV) // 800, 12800], i32, kind="ExternalOutput")
    out_s = nc.dram_tensor("out_s", [TPAD // 512, 512], f16, kind="ExternalOutput")

    ptab = nc.dram_tensor("ptab", [TPAD, 65], f16, addr_space="Shared")
    ptab_loc = nc.dram_tensor("ptab_loc", [IDS_PER_CORE, 65], f16)
    send = nc.dram_tensor("send", [SEND_ROWS, 64], f32)
    allh = nc.dram_tensor("allh", [NCORES * SEND_ROWS, 64], f32, addr_space="Shared")
    oq_loc = nc.dram_tensor("oq_loc", [IDS_PER_CORE, 64], i8)
    os_loc = nc.dram_tensor("os_loc", [IDS_PER_CORE, 1], f16)
    oq_full = nc.dram_tensor("oq_full", [TPAD, 64], i8, addr_space="Shared")
    os_full = nc.dram_tensor("os_full", [TPAD, 1], f16, addr_space="Shared")
    MAGIC = 12582912.0  # 1.5 * 2**23: (v + MAGIC) - MAGIC rounds v to nearest int

    with tile.TileContext(nc) as tc:
        with tc.tile_pool(name="const", bufs=1) as cp:
            ident = cp.tile([128, 128], f32)
            make_identity(nc, ident[:])
            iot_i = cp.tile([128, 128], i32)
            nc.gpsimd.iota(out=iot_i[:], pattern=[[1, 128]], base=0, channel_multiplier=0)
            iot_f = cp.tile([128, 128], f32)
            nc.vector.tensor_copy(out=iot_f[:], in_=iot_i[:])
            bt_sb = cp.tile([64, 64], f32)
            nc.sync.dma_start(out=bt_sb[:], in_=B_T[:])
            wv_sb = cp.tile([64, 64], f32)
            nc.sync.dma_start(out=wv_sb[:], in_=W_VT[:])
            wo_sb = cp.tile([65, 64], f32)
            nc.sync.dma_start(out=wo_sb[:], in_=W_oT[:])
            bo_sb = cp.tile([128, 64], f32)
            nc.sync.dma_start(out=bo_sb[:], in_=bo_rep[:])
            xo_sb = cp.tile([128, CPC], i32)
            nc.sync.dma_start(out=xo_sb[:], in_=xg_off[:])
            mo_sb = cp.tile([128, NBLK], i32)
            nc.sync.dma_start(out=mo_sb[:], in_=mrg_off[:])
            id8_sb = cp.tile([128, NBLK], mybir.dt.uint8)
            nc.sync.dma_start(out=id8_sb[:], in_=ids_u8[:])
            id_sb = cp.tile([128, NBLK], f32)
            nc.vector.tensor_copy(out=id_sb[:], in_=id8_sb[:])

            # ---------- exchange 0: rebuild the full node table on-device ----------
            # (collectives cannot read IO tensors: stage through Internal DRAM)
            nc.sync.dma_start(out=ptab_loc[:], in_=ptab_sh[:])
            nc.gpsimd.collective_compute(
                "AllGather", mybir.AluOpType.bypass,
                replica_groups=[list(range(NCORES))],
                ins=[ptab_loc[:]], outs=[ptab[:]])

            # ---------- phase A: per-cluster attention ----------
            with tc.tile_pool(name="asb", bufs=3) as asb, \
                 tc.tile_pool(name="aps", bufs=1, space="PSUM") as aps, \
                 tc.tile_pool(name="aps2", bufs=2, space="PSUM") as aps2, \
                 tc.tile_pool(name="xt4p", bufs=2) as xt4p, \
                 tc.tile_pool(name="xgp", bufs=6) as xgp:
                for g in range(CPC // 4):
                    XT4 = xt4p.tile([64, 512], f32)
                    xgs = []
                    for c4 in range(4):
                        c = g * 4 + c4
                        xgh = xgp.tile([128, 65], f16, tag="xgh")
                        nc.gpsimd.indirect_dma_start(
                            out=xgh[:, :], out_offset=None, in_=ptab[:],
                            in_offset=bass.IndirectOffsetOnAxis(ap=xo_sb[:, c:c + 1], axis=0))
                        xg = xgp.tile([128, 65], f32, tag="xg")
                        nc.vector.tensor_copy(out=xg[:], in_=xgh[:])
                        xgs.append(xg)
                        tp = aps.tile([64, 128], f32, tag="tp")
                        nc.tensor.transpose(out=tp[:], in_=xg[:, 0:64], identity=ident[:])
                        nc.any.tensor_copy(out=XT4[:, c4 * 128:(c4 + 1) * 128], in_=tp[:])
                    P4p = aps.tile([64, 512], f32, tag="p4")
                    nc.tensor.matmul(out=P4p[:], lhsT=bt_sb[:], rhs=XT4[:], start=True, stop=True)
                    P4 = asb.tile([64, 512], f32, tag="p4s")
                    nc.any.tensor_copy(out=P4[:], in_=P4p[:])
                    h4 = asb.tile([128, 4, 64], f32, tag="h4")
                    for c4 in range(4):
                        cs = slice(c4 * 128, (c4 + 1) * 128)
                        Vp = aps.tile([128, 64], f32, tag="vp")
                        nc.tensor.matmul(out=Vp[:], lhsT=XT4[:, cs], rhs=wv_sb[:], start=True, stop=True)
                        Vx = asb.tile([128, 65], f32, tag="vx")
                        nc.gpsimd.memset(Vx[:, 64:65], 1.0)
                        nc.any.tensor_copy(out=Vx[:, 0:64], in_=Vp[:])
                        STp = aps2.tile([128, 128], f32, tag="st")
                        nc.tensor.matmul(out=STp[:], lhsT=XT4[:, cs], rhs=P4[:, cs], start=True, stop=True)
                        y1 = asb.tile([128, 128], f32, tag="y1")
                        nc.vector.tensor_scalar(out=y1[:], in0=STp[:],
                                                scalar1=xgs[c4][:, 64:65], scalar2=None,
                                                op0=mybir.AluOpType.add)
                        y2 = asb.tile([128, 128], f32, tag="y2")
                        nc.vector.tensor_scalar(out=y2[:], in0=STp[:],
                                                scalar1=xgs[c4][:, 64:65], scalar2=0.2,
                                                op0=mybir.AluOpType.add,
                                                op1=mybir.AluOpType.mult)
                        L = asb.tile([128, 128], f32, tag="lr")
                        nc.vector.tensor_tensor(out=L[:], in0=y1[:], in1=y2[:],
                                                op=mybir.AluOpType.max)
                        E = asb.tile([128, 128], f32, tag="ex")
                        nc.scalar.activation(out=E[:], in_=L[:],
                                             func=mybir.ActivationFunctionType.Exp)
                        Hp = aps2.tile([128, 65], f32, tag="hp")
                        nc.tensor.matmul(out=Hp[:], lhsT=E[:], rhs=Vx[:], start=True, stop=True)
                        rec = asb.tile([128, 1], f32, tag="rec")
                        nc.vector.reciprocal(out=rec[:], in_=Hp[:, 64:65])
                        nc.vector.tensor_scalar_mul(h4[:, c4, :], Hp[:, 0:64], rec[:])
                    nc.sync.dma_start(
                        out=send[g * 512:(g + 1) * 512, :].rearrange("(c p) d -> p c d", p=128),
                        in_=h4[:, :, :])
                zz = asb.tile([128, 64], f32, tag="zz")
                nc.gpsimd.memset(zz[:], 0.0)
                nc.sync.dma_start(out=send[SEND_REAL:SEND_ROWS, :], in_=zz[:])

            # ---------- exchange 1: share per-cluster h rows ----------
            nc.gpsimd.collective_compute(
                "AllGather", mybir.AluOpType.bypass,
                replica_groups=[list(range(NCORES))],
                ins=[send[:]], outs=[allh[:]])

            # ---------- phase B: segment-sum + project + residual ----------
            with tc.tile_pool(name="bsb", bufs=4) as bsb, \
                 tc.tile_pool(name="bps", bufs=2, space="PSUM") as bps:
                for j in range(CHUNKS):
                    stgs = []
                    ohs = []
                    for w in range(BPC):
                        b = j * BPC + w
                        stg = bsb.tile([128, 65], f32, tag="stg")
                        nc.gpsimd.memset(stg[:, 64:65], 1.0)
                        nc.gpsimd.indirect_dma_start(
                            out=stg[:, 0:64], out_offset=None, in_=allh[:],
                            in_offset=bass.IndirectOffsetOnAxis(ap=mo_sb[:, b:b + 1], axis=0))
                        stgs.append(stg)
                        oh = bsb.tile([128, 128], f32, tag="oh")
                        eng = nc.vector
                        eng.tensor_tensor(out=oh[:], in0=id_sb[:, b:b + 1].to_broadcast([128, 128]),
                                          in1=iot_f[:], op=mybir.AluOpType.is_equal)
                        ohs.append(oh)
                    oT = bps.tile([65, 128], f32, tag="ot")
                    for w in range(BPC):
                        nc.tensor.matmul(out=oT[:], lhsT=stgs[w][:, :], rhs=ohs[w][:],
                                         start=(w == 0), stop=(w == BPC - 1))
                    cnat = bps.tile([128, 1], f32, tag="cn")
                    for w in range(BPC):
                        nc.tensor.matmul(out=cnat[:], lhsT=ohs[w][:], rhs=stgs[w][:, 64:65],
                                         start=(w == 0), stop=(w == BPC - 1))
                    oTs = bsb.tile([65, 128], f32, tag="ots")
                    nc.any.tensor_copy(out=oTs[:], in_=oT[:])
                    cm = bsb.tile([128, 1], f32, tag="cm")
                    nc.vector.tensor_scalar_max(cm[:], cnat[:], 1.0)
                    rc = bsb.tile([128, 1], f32, tag="rc")
                    nc.vector.reciprocal(out=rc[:], in_=cm[:])
                    fp = bps.tile([128, 64], f32, tag="fp")
                    nc.tensor.matmul(out=fp[:], lhsT=oTs[:], rhs=wo_sb[:], start=True, stop=True)
                    fs = bsb.tile([128, 64], f32, tag="fs")
                    nc.vector.tensor_scalar_mul(fs[:], fp[:], rc[:])
                    d = bsb.tile([128, 64], f32, tag="dlt")
                    nc.vector.tensor_add(out=d[:], in0=fs[:], in1=bo_sb[:])
                    # int8 quantization: q = round(d * 126/absmax), s = absmax/126
                    am = bsb.tile([128, 1], f32, tag="am")
                    nc.vector.tensor_reduce(out=am[:], in_=d[:],
                                            axis=mybir.AxisListType.X,
                                            op=mybir.AluOpType.max,
                                            apply_absolute_value=True)
                    am2 = bsb.tile([128, 1], f32, tag="am2")
                    nc.vector.tensor_scalar_max(am2[:], am[:], 1e-12)
                    qm = bsb.tile([128, 1], f32, tag="qm")
                    nc.vector.reciprocal(out=qm[:], in_=am2[:])
                    sc = bsb.tile([128, 1], f16, tag="sc")
                    nc.vector.tensor_scalar(out=sc[:], in0=am2[:], scalar1=1.0 / 126.0,
                                            scalar2=None, op0=mybir.AluOpType.mult)
                    qf = bsb.tile([128, 64], f32, tag="qf")
                    nc.vector.tensor_scalar(out=qf[:], in0=d[:], scalar1=qm[:],
                                            scalar2=126.0, op0=mybir.AluOpType.mult,
                                            op1=mybir.AluOpType.mult)
                    qr1 = bsb.tile([128, 64], f32, tag="qr1")
                    nc.vector.tensor_scalar(out=qr1[:], in0=qf[:], scalar1=MAGIC,
                                            scalar2=None, op0=mybir.AluOpType.add)
                    qr = bsb.tile([128, 64], f32, tag="qr")
                    nc.vector.tensor_scalar(out=qr[:], in0=qr1[:], scalar1=-MAGIC,
                                            scalar2=None, op0=mybir.AluOpType.add)
                    q8 = bsb.tile([128, 64], i8, tag="q8")
                    nc.vector.tensor_copy(out=q8[:], in_=qr[:])
                    nc.sync.dma_start(out=oq_loc[j * 128:(j + 1) * 128, :], in_=q8[:])
                    nc.sync.dma_start(out=os_loc[j * 128:(j + 1) * 128, :], in_=sc[:])

            # ---------- exchange 2: replicate the full output ----------
            nc.gpsimd.collective_compute(
                "AllGather", mybir.AluOpType.bypass,
                replica_groups=[list(range(NCORES))],
                ins=[oq_loc[:]], outs=[oq_full[:]])
            nc.gpsimd.collective_compute(
                "AllGather", mybir.AluOpType.bypass,
                replica_groups=[list(range(NCORES))],
                ins=[os_loc[:]], outs=[os_full[:]])
            nc.sync.dma_start(
                out=out_qa[:],
                in_=oq_full[0:NV, :].rearrange("(a b) d -> a (b d)", b=800).bitcast(i32))
            nc.sync.dma_start(
                out=out_qb[:],
                in_=oq_full[NV:NTOT, :].rearrange("(a b) d -> a (b d)", b=800).bitcast(i32))
            nc.sync.dma_start(
                out=out_s[:],
                in_=os_full[:].rearrange("(a b) o -> a (b o)", b=512))

    nc.compile()
    return nc


_ptab_buf = None


def _build_ptab(inputs):
    # Global (core-concatenated) table; shard i is rows [i*IDS_PER_CORE, ...)
    # Persistent buffer: pad rows and the var-side sat column stay zero, only
    # the data regions are rewritten each call.
    global _ptab_buf
    if _ptab_buf is None:
        _ptab_buf = np.zeros((TPAD, 65), np.float16)
    _ptab_buf[:NV, 0:64] = np.asarray(inputs["x_var"], np.float32)
    _ptab_buf[NV:NTOT, 0:64] = np.asarray(inputs["x_clause"], np.float32)
    _ptab_buf[NV:NTOT, 64] = GAMMA * np.asarray(inputs["satisfaction_scores"], np.float32)
    return _ptab_buf


def _prep(inputs):
    cvi = np.asarray(inputs["cluster_var_ids"]).astype(np.int64)
    cci = np.asarray(inputs["cluster_clause_ids"]).astype(np.int64)
    W_Q = np.asarray(inputs["W_Q"], np.float32)
    W_K = np.asarray(inputs["W_K"], np.float32)
    W_V = np.asarray(inputs["W_V"], np.float32)
    hww = np.asarray(inputs["head_weights"], np.float32)
    ah = int(inputs["active_heads"])
    Wo = np.asarray(inputs["out_proj_w"], np.float32)
    bo = np.asarray(inputs["out_proj_b"], np.float32)
    hw = float(np.mean(hww[:ah]))

    nodes = np.concatenate([cvi, cci + NV], 1).astype(np.int32)   # [2048, 128]

    B_Tm = (W_Q.T @ W_K / SCALE).astype(np.float32)
    W_VTm = (W_V * hw).T.copy().astype(np.float32)
    W_oTm = np.vstack([Wo.T, np.zeros((1, 64), np.float32)]).astype(np.float32)

    flat = nodes.reshape(-1).astype(np.int64)
    cidx = np.arange(C * 128) // 128
    slot = np.arange(C * 128) % 128
    allh_row = ((cidx // CPC) * SEND_ROWS + (cidx % CPC) * 128 + slot).astype(np.int64)
    order = np.argsort(flat, kind="stable")
    sids = flat[order]
    srows = allh_row[order].astype(np.int32)
    ZROW = SEND_REAL   # core 0's zero block

    bounds = np.searchsorted(sids, np.arange(0, TPAD + 128, 128))
    maxc = int(np.max(bounds[1:] - bounds[:-1]))
    BPC = 2 if maxc <= 256 else 3
    assert maxc <= BPC * 128, maxc
    S = BPC * 128
    NBLK = CHUNKS * BPC

    # flat scatter: element k of the sorted id list goes to global block
    # blk = sids//128 at within-block rank (k - bounds[blk])
    blk = (sids >> 7).astype(np.int64)
    within = np.arange(sids.shape[0], dtype=np.int64) - bounds[blk]
    dest = blk * S + within
    mrg_flat = np.full((NCORES * CHUNKS * S,), ZROW, np.int32)
    ids_flat = np.full((NCORES * CHUNKS * S,), 255, np.uint8)  # 255 = masked pad
    mrg_flat[dest] = srows
    ids_flat[dest] = (sids & 127).astype(np.uint8)

    xg_all = np.ascontiguousarray(
        nodes.reshape(NCORES, CPC, 128).transpose(0, 2, 1))
    mrg_all = np.ascontiguousarray(
        mrg_flat.reshape(NCORES, NBLK, 128).transpose(0, 2, 1))
    ids_all = np.ascontiguousarray(
        ids_flat.reshape(NCORES, NBLK, 128).transpose(0, 2, 1))

    rep = lambda a: np.ascontiguousarray(np.broadcast_to(a, (NCORES,) + a.shape)
                                         ).reshape(NCORES * a.shape[0], *a.shape[1:])
    globals_map = {
        "xg_off": xg_all.reshape(NCORES * 128, CPC),
        "mrg_off": mrg_all.reshape(NCORES * 128, NBLK),
        "ids_u8": ids_all.reshape(NCORES * 128, NBLK),
        "B_T": rep(B_Tm),
        "W_VT": rep(W_VTm),
        "W_oT": rep(W_oTm),
        "bo_rep": rep(np.ascontiguousarray(np.broadcast_to(bo, (128, 64)), np.float32)),
    }
    return globals_map, BPC


class _Runner:
    """Cached jitted SPMD dispatch for a compiled Bass module (one trace,
    device-side output zero buffers, global inputs laid out so the per-core
    shard is exactly the BIR-declared shape with no host concat)."""

    def __init__(self, nc):
        import jax
        import concourse.mybir as mybir
        from concourse.bass2jax import (_bass_exec_p, install_neuronx_cc_hook,
                                        partition_id_tensor)
        from jax.sharding import Mesh, PartitionSpec, NamedSharding
        from jax.experimental.shard_map import shard_map

        install_neuronx_cc_hook()
        self.jax = jax
        partition_name = nc.partition_id_tensor.name if nc.partition_id_tensor else None
        in_names, out_names, out_avals, out_shapes = [], [], [], []
        for alloc in nc.m.functions[0].allocations:
            if not isinstance(alloc, mybir.MemoryLocationSet):
                continue
            name = alloc.memorylocations[0].name
            if alloc.kind == "ExternalInput":
                if name != partition_name:
                    in_names.append(name)
            elif alloc.kind == "ExternalOutput":
                shape = tuple(alloc.tensor_shape)
                dtype = mybir.dt.np(alloc.dtype)
                out_names.append(name)
                out_avals.append(jax.core.ShapedArray(shape, dtype))
                out_shapes.append((shape, dtype))
        n_params = len(in_names)
        n_outs = len(out_avals)
        all_names = list(in_names) + list(out_names)
        if partition_name is not None:
            all_names.append(partition_name)
        self.in_names = in_names
        self.out_names = out_names

        def _body(*args):
            operands = list(args)
            if partition_name is not None:
                operands.append(partition_id_tensor())
            outs = _bass_exec_p.bind(
                *operands,
                out_avals=tuple(out_avals),
                in_names=tuple(all_names),
                out_names=tuple(out_names),
                lowering_input_output_aliases=(),
                sim_require_finite=True,
                sim_require_nnan=True,
                nc=nc,
            )
            return tuple(outs)

        devices = jax.devices()[:NCORES]
        assert len(devices) == NCORES
        mesh = Mesh(np.asarray(devices), ("core",))
        spec = NamedSharding(mesh, PartitionSpec("core"))
        self._spec = spec
        in_specs = (PartitionSpec("core"),) * (n_params + n_outs)
        # every output is replicated by the kernel's final AllGather, so jax
        # only fetches it from a single device
        out_specs = (PartitionSpec(),) * n_outs
        self._sharded = jax.jit(
            shard_map(_body, mesh=mesh, in_specs=in_specs, out_specs=out_specs,
                      check_rep=False),
            keep_unused=True)

        def _mk():
            import jax.numpy as jnp
            return tuple(jnp.zeros((NCORES * s[0],) + tuple(s[1:]), d)
                         for s, d in out_shapes)
        # the NEFF writes every output element, so the per-call content of the
        # donate-style out buffers is irrelevant: build them once and reuse
        self._zeros = jax.block_until_ready(
            jax.jit(_mk, out_shardings=(spec,) * n_outs)())

    def put_small(self, globals_map):
        # async H2D of the small per-call tables; overlaps ptab fp16 build
        return {n: self.jax.device_put(a, self._spec)
                for n, a in globals_map.items()}

    def __call__(self, d_small, ptab):
        ins = [ptab if n == "ptab_sh" else d_small[n] for n in self.in_names]
        outs = self._sharded(*ins, *self._zeros)
        return {n: outs[i] for i, n in enumerate(self.out_names)}


def run(inputs):
    globals_map, BPC = _prep(inputs)
    if BPC not in _cache:
        nc = _build(BPC)
        _cache[BPC] = _Runner(nc)
    runner = _cache[BPC]
    d_small = runner.put_small(globals_map)   # async H2D
    ptab = _build_ptab(inputs)                # overlaps the small transfers
    outs = runner(d_small, ptab)
    # pipelined D2H: fetch the clause half in a background thread (the
    # device-to-host copy releases the GIL) while reconstructing the var half
    import threading
    s32 = np.asarray(outs["out_s"]).reshape(TPAD, 1)[:NTOT].astype(np.float32)
    res = {}

    def _fetch_b():
        res["qb"] = np.asarray(outs["out_qb"])

    qa = np.asarray(outs["out_qa"])
    tb = threading.Thread(target=_fetch_b)
    tb.start()
    var = qa.view(np.int8).reshape(NV, 64).astype(np.float32)
    var *= s32[:NV]
    var += np.asarray(inputs["x_var"], np.float32)
    tb.join()
    clause = res["qb"].view(np.int8).reshape(NTOT - NV, 64).astype(np.float32)
    clause *= s32[NV:]
    clause += np.asarray(inputs["x_clause"], np.float32)
    return (var, clause)


def kernel(**inputs):
    return run(inputs)


# revision 42
# speedup vs baseline: 1.3387x; 1.3387x over previous
import numpy as np

NV = 100000
NTOT = 200000
C = 2048
CPC = 256            # clusters per core
NCORES = 8
CHUNKS = 196         # output chunks of 128 ids per core
IDS_PER_CORE = CHUNKS * 128          # 25088
TPAD = NCORES * IDS_PER_CORE         # 200704 padded id space
SEND_REAL = CPC * 128                # 32768 h rows per core
SEND_ROWS = SEND_REAL + 128          # + zero block
GAMMA = 1.0
SCALE = 8.0          # sqrt(64)

_cache = {}


def _build(BPC):
    import concourse.bass as bass
    import concourse.mybir as mybir
    import concourse.tile as tile
    import concourse.bacc as bacc
    from concourse.masks import make_identity

    f32 = mybir.dt.float32
    f16 = mybir.dt.float16
    i32 = mybir.dt.int32
    NBLK = CHUNKS * BPC

    nc = bacc.Bacc("TRN2", target_bir_lowering=False, debug=False)
    ptab_sh = nc.dram_tensor("ptab_sh", [IDS_PER_CORE, 65], f16, kind="ExternalInput")
    xg_off = nc.dram_tensor("xg_off", [128, CPC], i32, kind="ExternalInput")
    mrg_off = nc.dram_tensor("mrg_off", [128, NBLK], i32, kind="ExternalInput")
    ids_u8 = nc.dram_tensor("ids_u8", [128, NBLK], mybir.dt.uint8, kind="ExternalInput")
    B_T = nc.dram_tensor("B_T", [64, 64], f32, kind="ExternalInput")
    W_VT = nc.dram_tensor("W_VT", [64, 64], f32, kind="ExternalInput")
    W_oT = nc.dram_tensor("W_oT", [65, 64], f32, kind="ExternalInput")
    bo_rep = nc.dram_tensor("bo_rep", [128, 64], f32, kind="ExternalInput")
    # outputs: int8 delta + per-row fp16 scale, replicated on every core by a
    # final AllGather so the host fetches one single-stream copy (per-shard
    # D2H has heavy overhead). Host reconstructs x + q*scale in f32.
    # wide-row, 4-byte-dtype output shapes: the tunnel's D2H throughput is
    # per-element, so ship the int8/f16 payload bitcast as int32/f32
    i8 = mybir.dt.int8
    out_qa = nc.dram_tensor("out_qa", [NV // 800, 12800], i32, kind="ExternalOutput")
    out_qb = nc.dram_tensor("out_qb", [(NTOT - NV) // 800, 12800], i32, kind="ExternalOutput")
    out_s = nc.dram_tensor("out_s", [TPAD // 512, 512], f16, kind="ExternalOutput")

    ptab = nc.dram_tensor("ptab", [TPAD, 65], f16, addr_space="Shared")
    ptab_loc = nc.dram_tensor("ptab_loc", [IDS_PER_CORE, 65], f16)
    send = nc.dram_tensor("send", [SEND_ROWS, 64], f32)
    allh = nc.dram_tensor("allh", [NCORES * SEND_ROWS, 64], f32, addr_space="Shared")
    oq_loc = nc.dram_tensor("oq_loc", [IDS_PER_CORE, 64], i8)
    os_loc = nc.dram_tensor("os_loc", [IDS_PER_CORE, 1], f16)
    oq_full = nc.dram_tensor("oq_full", [TPAD, 64], i8, addr_space="Shared")
    os_full = nc.dram_tensor("os_full", [TPAD, 1], f16, addr_space="Shared")
    MAGIC = 12582912.0  # 1.5 * 2**23: (v + MAGIC) - MAGIC rounds v to nearest int

    with tile.TileContext(nc) as tc:
        with tc.tile_pool(name="const", bufs=1) as cp:
            ident = cp.tile([128, 128], f32)
            make_identity(nc, ident[:])
            iot_i = cp.tile([128, 128], i32)
            nc.gpsimd.iota(out=iot_i[:], pattern=[[1, 128]], base=0, channel_multiplier=0)
            iot_f = cp.tile([128, 128], f32)
            nc.vector.tensor_copy(out=iot_f[:], in_=iot_i[:])
            bt_sb = cp.tile([64, 64], f32)
            nc.sync.dma_start(out=bt_sb[:], in_=B_T[:])
            wv_sb = cp.tile([64, 64], f32)
            nc.sync.dma_start(out=wv_sb[:], in_=W_VT[:])
            wo_sb = cp.tile([65, 64], f32)
            nc.sync.dma_start(out=wo_sb[:], in_=W_oT[:])
            bo_sb = cp.tile([128, 64], f32)
            nc.sync.dma_start(out=bo_sb[:], in_=bo_rep[:])
            xo_sb = cp.tile([128, CPC], i32)
            nc.sync.dma_start(out=xo_sb[:], in_=xg_off[:])
            mo_sb = cp.tile([128, NBLK], i32)
            nc.sync.dma_start(out=mo_sb[:], in_=mrg_off[:])
            id8_sb = cp.tile([128, NBLK], mybir.dt.uint8)
            nc.sync.dma_start(out=id8_sb[:], in_=ids_u8[:])
            id_sb = cp.tile([128, NBLK], f32)
            nc.vector.tensor_copy(out=id_sb[:], in_=id8_sb[:])

            # ---------- exchange 0: rebuild the full node table on-device ----------
            # (collectives cannot read IO tensors: stage through Internal DRAM)
            nc.sync.dma_start(out=ptab_loc[:], in_=ptab_sh[:])
            nc.gpsimd.collective_compute(
                "AllGather", mybir.AluOpType.bypass,
                replica_groups=[list(range(NCORES))],
                ins=[ptab_loc[:]], outs=[ptab[:]])

            # ---------- phase A: per-cluster attention ----------
            with tc.tile_pool(name="asb", bufs=3) as asb, \
                 tc.tile_pool(name="aps", bufs=1, space="PSUM") as aps, \
                 tc.tile_pool(name="aps2", bufs=2, space="PSUM") as aps2, \
                 tc.tile_pool(name="xt4p", bufs=2) as xt4p, \
                 tc.tile_pool(name="xgp", bufs=6) as xgp:
                for g in range(CPC // 4):
                    XT4 = xt4p.tile([64, 512], f32)
                    xgs = []
                    for c4 in range(4):
                        c = g * 4 + c4
                        xgh = xgp.tile([128, 65], f16, tag="xgh")
                        nc.gpsimd.indirect_dma_start(
                            out=xgh[:, :], out_offset=None, in_=ptab[:],
                            in_offset=bass.IndirectOffsetOnAxis(ap=xo_sb[:, c:c + 1], axis=0))
                        xg = xgp.tile([128, 65], f32, tag="xg")
                        nc.vector.tensor_copy(out=xg[:], in_=xgh[:])
                        xgs.append(xg)
                        tp = aps.tile([64, 128], f32, tag="tp")
                        nc.tensor.transpose(out=tp[:], in_=xg[:, 0:64], identity=ident[:])
                        nc.any.tensor_copy(out=XT4[:, c4 * 128:(c4 + 1) * 128], in_=tp[:])
                    P4p = aps.tile([64, 512], f32, tag="p4")
                    nc.tensor.matmul(out=P4p[:], lhsT=bt_sb[:], rhs=XT4[:], start=True, stop=True)
                    P4 = asb.tile([64, 512], f32, tag="p4s")
                    nc.any.tensor_copy(out=P4[:], in_=P4p[:])
                    h4 = asb.tile([128, 4, 64], f32, tag="h4")
                    for c4 in range(4):
                        cs = slice(c4 * 128, (c4 + 1) * 128)
                        Vp = aps.tile([128, 64], f32, tag="vp")
                        nc.tensor.matmul(out=Vp[:], lhsT=XT4[:, cs], rhs=wv_sb[:], start=True, stop=True)
                        Vx = asb.tile([128, 65], f32, tag="vx")
                        nc.gpsimd.memset(Vx[:, 64:65], 1.0)
                        nc.any.tensor_copy(out=Vx[:, 0:64], in_=Vp[:])
                        STp = aps2.tile([128, 128], f32, tag="st")
                        nc.tensor.matmul(out=STp[:], lhsT=XT4[:, cs], rhs=P4[:, cs], start=True, stop=True)
                        y1 = asb.tile([128, 128], f32, tag="y1")
                        nc.vector.tensor_scalar(out=y1[:], in0=STp[:],
                                                scalar1=xgs[c4][:, 64:65], scalar2=None,
                                                op0=mybir.AluOpType.add)
                        y2 = asb.tile([128, 128], f32, tag="y2")
                        nc.vector.tensor_scalar(out=y2[:], in0=STp[:],
                                                scalar1=xgs[c4][:, 64:65], scalar2=0.2,
                                                op0=mybir.AluOpType.add,
                                                op1=mybir.AluOpType.mult)
                        L = asb.tile([128, 128], f32, tag="lr")
                        nc.vector.tensor_tensor(out=L[:], in0=y1[:], in1=y2[:],
                                                op=mybir.AluOpType.max)
                        E = asb.tile([128, 128], f32, tag="ex")
                        nc.scalar.activation(out=E[:], in_=L[:],
                                             func=mybir.ActivationFunctionType.Exp)
                        Hp = aps2.tile([128, 65], f32, tag="hp")
                        nc.tensor.matmul(out=Hp[:], lhsT=E[:], rhs=Vx[:], start=True, stop=True)
                        rec = asb.tile([128, 1], f32, tag="rec")
                        nc.vector.reciprocal(out=rec[:], in_=Hp[:, 64:65])
                        nc.vector.tensor_scalar_mul(h4[:, c4, :], Hp[:, 0:64], rec[:])
                    nc.sync.dma_start(
                        out=send[g * 512:(g + 1) * 512, :].rearrange("(c p) d -> p c d", p=128),
                        in_=h4[:, :, :])
                zz = asb.tile([128, 64], f32, tag="zz")
                nc.gpsimd.memset(zz[:], 0.0)
                nc.sync.dma_start(out=send[SEND_REAL:SEND_ROWS, :], in_=zz[:])

            # ---------- exchange 1: share per-cluster h rows ----------
            nc.gpsimd.collective_compute(
                "AllGather", mybir.AluOpType.bypass,
                replica_groups=[list(range(NCORES))],
                ins=[send[:]], outs=[allh[:]])

            # ---------- phase B: segment-sum + project + residual ----------
            with tc.tile_pool(name="bsb", bufs=4) as bsb, \
                 tc.tile_pool(name="bps", bufs=2, space="PSUM") as bps:
                for j in range(CHUNKS):
                    stgs = []
                    ohs = []
                    for w in range(BPC):
                        b = j * BPC + w
                        stg = bsb.tile([128, 65], f32, tag="stg")
                        nc.gpsimd.memset(stg[:, 64:65], 1.0)
                        nc.gpsimd.indirect_dma_start(
                            out=stg[:, 0:64], out_offset=None, in_=allh[:],
                            in_offset=bass.IndirectOffsetOnAxis(ap=mo_sb[:, b:b + 1], axis=0))
                        stgs.append(stg)
                        oh = bsb.tile([128, 128], f32, tag="oh")
                        eng = nc.vector
                        eng.tensor_tensor(out=oh[:], in0=id_sb[:, b:b + 1].to_broadcast([128, 128]),
                                          in1=iot_f[:], op=mybir.AluOpType.is_equal)
                        ohs.append(oh)
                    oT = bps.tile([65, 128], f32, tag="ot")
                    for w in range(BPC):
                        nc.tensor.matmul(out=oT[:], lhsT=stgs[w][:, :], rhs=ohs[w][:],
                                         start=(w == 0), stop=(w == BPC - 1))
                    cnat = bps.tile([128, 1], f32, tag="cn")
                    for w in range(BPC):
                        nc.tensor.matmul(out=cnat[:], lhsT=ohs[w][:], rhs=stgs[w][:, 64:65],
                                         start=(w == 0), stop=(w == BPC - 1))
                    oTs = bsb.tile([65, 128], f32, tag="ots")
                    nc.any.tensor_copy(out=oTs[:], in_=oT[:])
                    cm = bsb.tile([128, 1], f32, tag="cm")
                    nc.vector.tensor_scalar_max(cm[:], cnat[:], 1.0)
                    rc = bsb.tile([128, 1], f32, tag="rc")
                    nc.vector.reciprocal(out=rc[:], in_=cm[:])
                    fp = bps.tile([128, 64], f32, tag="fp")
                    nc.tensor.matmul(out=fp[:], lhsT=oTs[:], rhs=wo_sb[:], start=True, stop=True)
                    fs = bsb.tile([128, 64], f32, tag="fs")
                    nc.vector.tensor_scalar_mul(fs[:], fp[:], rc[:])
                    d = bsb.tile([128, 64], f32, tag="dlt")
                    nc.vector.tensor_add(out=d[:], in0=fs[:], in1=bo_sb[:])
                    # int8 quantization: q = round(d * 126/absmax), s = absmax/126
                    am = bsb.tile([128, 1], f32, tag="am")
                    nc.vector.tensor_reduce(out=am[:], in_=d[:],
                                            axis=mybir.AxisListType.X,
                                            op=mybir.AluOpType.max,
                                            apply_absolute_value=True)
                    am2 = bsb.tile([128, 1], f32, tag="am2")
                    nc.vector.tensor_scalar_max(am2[:], am[:], 1e-12)
                    qm = bsb.tile([128, 1], f32, tag="qm")
                    nc.vector.reciprocal(out=qm[:], in_=am2[:])
                    sc = bsb.tile([128, 1], f16, tag="sc")
                    nc.vector.tensor_scalar(out=sc[:], in0=am2[:], scalar1=1.0 / 126.0,
                                            scalar2=None, op0=mybir.AluOpType.mult)
                    qf = bsb.tile([128, 64], f32, tag="qf")
                    nc.vector.tensor_scalar(out=qf[:], in0=d[:], scalar1=qm[:],
                                            scalar2=126.0, op0=mybir.AluOpType.mult,
                                            op1=mybir.AluOpType.mult)
                    qr1 = bsb.tile([128, 64], f32, tag="qr1")
                    nc.vector.tensor_scalar(out=qr1[:], in0=qf[:], scalar1=MAGIC,
                                            scalar2=None, op0=mybir.AluOpType.add)
                    qr = bsb.tile([128, 64], f32, tag="qr")
                    nc.vector.tensor_scalar(out=qr[:], in0=qr1[:], scalar1=-MAGIC,
                                            scalar2=None, op0=mybir.AluOpType.add)
                    q8 = bsb.tile([128, 64], i8, tag="q8")
                    nc.vector.tensor_copy(out=q8[:], in_=qr[:])
                    nc.sync.dma_start(out=oq_loc[j * 128:(j + 1) * 128, :], in_=q8[:])
                    nc.sync.dma_start(out=os_loc[j * 128:(j + 1) * 128, :], in_=sc[:])

            # ---------- exchange 2: replicate the full output ----------
            nc.gpsimd.collective_compute(
                "AllGather", mybir.AluOpType.bypass,
                replica_groups=[list(range(NCORES))],
                ins=[oq_loc[:]], outs=[oq_full[:]])
            nc.gpsimd.collective_compute(
                "AllGather", mybir.AluOpType.bypass,
                replica_groups=[list(range(NCORES))],
                ins=[os_loc[:]], outs=[os_full[:]])
            nc.sync.dma_start(
                out=out_qa[:],
                in_=oq_full[0:NV, :].rearrange("(a b) d -> a (b d)", b=800).bitcast(i32))
            nc.sync.dma_start(
                out=out_qb[:],
                in_=oq_full[NV:NTOT, :].rearrange("(a b) d -> a (b d)", b=800).bitcast(i32))
            nc.sync.dma_start(
                out=out_s[:],
                in_=os_full[:].rearrange("(a b) o -> a (b o)", b=512))

    nc.compile()
    return nc


_ptab_buf = None


def _build_ptab(inputs):
    # Global (core-concatenated) table; shard i is rows [i*IDS_PER_CORE, ...)
    # Persistent buffer: pad rows and the var-side sat column stay zero, only
    # the data regions are rewritten each call.
    global _ptab_buf
    if _ptab_buf is None:
        _ptab_buf = np.zeros((TPAD, 65), np.float16)
    _ptab_buf[:NV, 0:64] = np.asarray(inputs["x_var"], np.float32)
    _ptab_buf[NV:NTOT, 0:64] = np.asarray(inputs["x_clause"], np.float32)
    _ptab_buf[NV:NTOT, 64] = GAMMA * np.asarray(inputs["satisfaction_scores"], np.float32)
    return _ptab_buf


def _prep(inputs):
    cvi = np.asarray(inputs["cluster_var_ids"]).astype(np.int64)
    cci = np.asarray(inputs["cluster_clause_ids"]).astype(np.int64)
    W_Q = np.asarray(inputs["W_Q"], np.float32)
    W_K = np.asarray(inputs["W_K"], np.float32)
    W_V = np.asarray(inputs["W_V"], np.float32)
    hww = np.asarray(inputs["head_weights"], np.float32)
    ah = int(inputs["active_heads"])
    Wo = np.asarray(inputs["out_proj_w"], np.float32)
    bo = np.asarray(inputs["out_proj_b"], np.float32)
    hw = float(np.mean(hww[:ah]))

    nodes = np.concatenate([cvi, cci + NV], 1).astype(np.int32)   # [2048, 128]

    B_Tm = (W_Q.T @ W_K / SCALE).astype(np.float32)
    W_VTm = (W_V * hw).T.copy().astype(np.float32)
    W_oTm = np.vstack([Wo.T, np.zeros((1, 64), np.float32)]).astype(np.float32)

    flat = nodes.reshape(-1).astype(np.int64)
    cidx = np.arange(C * 128) // 128
    slot = np.arange(C * 128) % 128
    allh_row = ((cidx // CPC) * SEND_ROWS + (cidx % CPC) * 128 + slot).astype(np.int64)
    order = np.argsort(flat, kind="stable")
    sids = flat[order]
    srows = allh_row[order].astype(np.int32)
    ZROW = SEND_REAL   # core 0's zero block

    bounds = np.searchsorted(sids, np.arange(0, TPAD + 128, 128))
    maxc = int(np.max(bounds[1:] - bounds[:-1]))
    BPC = 2 if maxc <= 256 else 3
    assert maxc <= BPC * 128, maxc
    S = BPC * 128
    NBLK = CHUNKS * BPC

    # flat scatter: element k of the sorted id list goes to global block
    # blk = sids//128 at within-block rank (k - bounds[blk])
    blk = (sids >> 7).astype(np.int64)
    within = np.arange(sids.shape[0], dtype=np.int64) - bounds[blk]
    dest = blk * S + within
    mrg_flat = np.full((NCORES * CHUNKS * S,), ZROW, np.int32)
    ids_flat = np.full((NCORES * CHUNKS * S,), 255, np.uint8)  # 255 = masked pad
    mrg_flat[dest] = srows
    ids_flat[dest] = (sids & 127).astype(np.uint8)

    xg_all = np.ascontiguousarray(
        nodes.reshape(NCORES, CPC, 128).transpose(0, 2, 1))
    mrg_all = np.ascontiguousarray(
        mrg_flat.reshape(NCORES, NBLK, 128).transpose(0, 2, 1))
    ids_all = np.ascontiguousarray(
        ids_flat.reshape(NCORES, NBLK, 128).transpose(0, 2, 1))

    rep = lambda a: np.ascontiguousarray(np.broadcast_to(a, (NCORES,) + a.shape)
                                         ).reshape(NCORES * a.shape[0], *a.shape[1:])
    globals_map = {
        "xg_off": xg_all.reshape(NCORES * 128, CPC),
        "mrg_off": mrg_all.reshape(NCORES * 128, NBLK),
        "ids_u8": ids_all.reshape(NCORES * 128, NBLK),
        "B_T": rep(B_Tm),
        "W_VT": rep(W_VTm),
        "W_oT": rep(W_oTm),
        "bo_rep": rep(np.ascontiguousarray(np.broadcast_to(bo, (128, 64)), np.float32)),
    }
    return globals_map, BPC


class _Runner:
    """Cached jitted SPMD dispatch for a compiled Bass module (one trace,
    device-side output zero buffers, global inputs laid out so the per-core
    shard is exactly the BIR-declared shape with no host concat)."""

    def __init__(self, nc):
        import jax
        import concourse.mybir as mybir
        from concourse.bass2jax import (_bass_exec_p, install_neuronx_cc_hook,
                                        partition_id_tensor)
        from jax.sharding import Mesh, PartitionSpec, NamedSharding
        from jax.experimental.shard_map import shard_map

        install_neuronx_cc_hook()
        self.jax = jax
        partition_name = nc.partition_id_tensor.name if nc.partition_id_tensor else None
        in_names, out_names, out_avals, out_shapes = [], [], [], []
        for alloc in nc.m.functions[0].allocations:
            if not isinstance(alloc, mybir.MemoryLocationSet):
                continue
            name = alloc.memorylocations[0].name
            if alloc.kind == "ExternalInput":
                if name != partition_name:
                    in_names.append(name)
            elif alloc.kind == "ExternalOutput":
                shape = tuple(alloc.tensor_shape)
                dtype = mybir.dt.np(alloc.dtype)
                out_names.append(name)
                out_avals.append(jax.core.ShapedArray(shape, dtype))
                out_shapes.append((shape, dtype))
        n_params = len(in_names)
        n_outs = len(out_avals)
        all_names = list(in_names) + list(out_names)
        if partition_name is not None:
            all_names.append(partition_name)
        self.in_names = in_names
        self.out_names = out_names

        def _body(*args):
            operands = list(args)
            if partition_name is not None:
                operands.append(partition_id_tensor())
            outs = _bass_exec_p.bind(
                *operands,
                out_avals=tuple(out_avals),
                in_names=tuple(all_names),
                out_names=tuple(out_names),
                lowering_input_output_aliases=(),
                sim_require_finite=True,
                sim_require_nnan=True,
                nc=nc,
            )
            return tuple(outs)

        devices = jax.devices()[:NCORES]
        assert len(devices) == NCORES
        mesh = Mesh(np.asarray(devices), ("core",))
        spec = NamedSharding(mesh, PartitionSpec("core"))
        self._spec = spec
        in_specs = (PartitionSpec("core"),) * (n_params + n_outs)
        # every output is replicated by the kernel's final AllGather, so jax
        # only fetches it from a single device
        out_specs = (PartitionSpec(),) * n_outs
        self._sharded = jax.jit(
            shard_map(_body, mesh=mesh, in_specs=in_specs, out_specs=out_specs,
                      check_rep=False),
            keep_unused=True)

        def _mk():
            import jax.numpy as jnp
            return tuple(jnp.zeros((NCORES * s[0],) + tuple(s[1:]), d)
                         for s, d in out_shapes)
        # the NEFF writes every output element, so the per-call content of the
        # donate-style out buffers is irrelevant: build them once and reuse
        self._zeros = jax.block_until_ready(
            jax.jit(_mk, out_shardings=(spec,) * n_outs)())

    def put_small(self, globals_map):
        # async H2D of the small per-call tables; overlaps ptab fp16 build
        return {n: self.jax.device_put(a, self._spec)
                for n, a in globals_map.items()}

    def __call__(self, d_small, ptab):
        ins = [ptab if n == "ptab_sh" else d_small[n] for n in self.in_names]
        outs = self._sharded(*ins, *self._zeros)
        return {n: outs[i] for i, n in enumerate(self.out_names)}


def run(inputs):
    globals_map, BPC = _prep(inputs)
    if BPC not in _cache:
        nc = _build(BPC)
        _cache[BPC] = _Runner(nc)
    runner = _cache[BPC]
    d_small = runner.put_small(globals_map)   # async H2D
    ptab = _build_ptab(inputs)                # overlaps the small transfers
    outs = runner(d_small, ptab)
    # pipelined D2H: fetch the clause half in a background thread (the
    # device-to-host copy releases the GIL) while reconstructing the var half
    import threading
    res = {}

    def _fetch(name):
        res[name] = np.asarray(outs[name])

    threads = [threading.Thread(target=_fetch, args=(n,))
               for n in ("out_s", "out_qa", "out_qb")]
    for t in threads:
        t.start()
    threads[0].join()
    s32 = res["out_s"].reshape(TPAD, 1)[:NTOT].astype(np.float32)
    threads[1].join()
    var = res["out_qa"].view(np.int8).reshape(NV, 64).astype(np.float32)
    var *= s32[:NV]
    var += np.asarray(inputs["x_var"], np.float32)
    threads[2].join()
    clause = res["out_qb"].view(np.int8).reshape(NTOT - NV, 64).astype(np.float32)
    clause *= s32[NV:]
    clause += np.asarray(inputs["x_clause"], np.float32)
    return (var, clause)


def kernel(**inputs):
    return run(inputs)
